# revision 41
# baseline (speedup 1.0000x reference)
"""BinarySelfAttention Trainium2 kernel (8-core SPMD).

Strategy: shard (batch, head-group): core c -> batch c//4, heads 4*(c%4)..+3.
Each core computes ternary-projected QKV for its 4 heads, RoPE, causal
flash-style attention in S^T orientation (keys on partitions -> no transposes),
and a partial output projection against its Wo column slice. Host sums the 4
partials per batch.

All matmuls run in float32r (TF32-like, full PE rate at moving-dim >= 256).
Ternary weight signs {-1,0,1} are exact in f32r; ternary scales are folded
into the exp() scale (sq*sk/8) and the final output eviction (sv*so), both
passed as runtime data so the compiled program is input-independent.
"""
import numpy as np

import concourse.bass as bass
import concourse.mybir as mybir
import concourse.tile as tile
from concourse.bass_utils import run_bass_kernel_spmd
from concourse.tile_rust import add_dep_helper

F32 = mybir.dt.float32
F32R = mybir.dt.float32r

B, T, D, H = 2, 2048, 1024, 16
HD = 64            # head dim
HPC = 4            # heads per core
FPC = HPC * HD     # features per core (256)
NCORES = 8
KC = D // 128      # 8 contraction chunks for projections


def _split_excess_waits(nc, max_waits=1):
    """TRN2 ISA has one sem-wait slot per instruction and this walrus build
    rejects 3+; hoist excess waits onto preceding same-engine NOPs."""
    n = 0
    for f in nc.m.functions:
        for bb in f.blocks:
            new_insts = []
            for inst in bb.instructions:
                si = getattr(inst, 'sync_info', None)
                if si is not None and si.on_wait and len(si.on_wait) > max_waits:
                    waits = list(si.on_wait)
                    extra, keep = waits[:-max_waits], waits[-max_waits:]
                    for j, w in enumerate(extra):
                        new_insts.append(mybir.InstNoOp(
                            name=f"{inst.name}-wsplit{j}",
                            engine=inst.engine,
                            sync_info=mybir.SyncInfo(on_wait=[w], on_update=[]),
                            bass_nofuse=True,
                        ))
                        n += 1
                    inst.sync_info = mybir.SyncInfo(
                        on_wait=keep, on_update=si.on_update)
                new_insts.append(inst)
            bb.instructions[:] = new_insts
    return n


def _build():
    nc = bass.Bass("TRN2", target_bir_lowering=False, debug=False,
                   num_devices=NCORES)
    xt_d = nc.dram_tensor("xt", [D, T], F32R, kind="ExternalInput")
    wq_d = nc.dram_tensor("wqt", [D, FPC], F32R, kind="ExternalInput")
    wk_d = nc.dram_tensor("wkt", [D, FPC], F32R, kind="ExternalInput")
    wv_d = nc.dram_tensor("wvt", [D, FPC], F32R, kind="ExternalInput")
    wo_d = nc.dram_tensor("woc", [FPC, D], F32R, kind="ExternalInput")
    cos_d = nc.dram_tensor("cos2", [128, T], F32, kind="ExternalInput")
    sin_d = nc.dram_tensor("sins", [128, T], F32, kind="ExternalInput")
    msk_d = nc.dram_tensor("maskm", [128, 128], F32R, kind="ExternalInput")
    con_d = nc.dram_tensor("consts", [128, 2], F32, kind="ExternalInput")
    yp_d = nc.dram_tensor("yp", [T, D], F32, kind="ExternalOutput")
    rec_d = nc.dram_tensor("recd", [HPC, T], F32)  # internal scratch

    EXP = mybir.ActivationFunctionType.Exp
    CPY = mybir.ActivationFunctionType.Copy

    with tile.TileContext(nc) as tc:
        with tc.tile_pool(name="main", bufs=1) as mp:
            CON = mp.tile([128, 2], F32)
            MSK = mp.tile([128, 128], F32R)
            QT = [mp.tile([128, T], F32R, tag=f"qt{i}", name=f"qt{i}") for i in range(2)]
            KT = [mp.tile([128, T], F32R, tag=f"kt{i}", name=f"kt{i}") for i in range(2)]
            VA = mp.tile([128, 16, HPC * 65], F32R)
            ONES = mp.tile([128, 64], F32)

            nc.sync.dma_start(out=CON, in_=con_d[:, :])
            nc.sync.dma_start(out=MSK, in_=msk_d[:, :])
            nc.vector.memset(ONES, 1.0)
            ones_view = VA[:, :, :].rearrange(
                "p a (h e) -> p a h e", e=65)[:, :, :, 64:65].rearrange(
                "p a h e -> p (a h e)")
            nc.vector.tensor_copy(out=ones_view, in_=ONES[:, 0:64])

            # ---------------- Phase 1: projections + RoPE ----------------
            ptp_cm = tc.tile_pool(name="pt", bufs=7)
            ptp = ptp_cm.__enter__()
            with tc.tile_pool(name="p1", bufs=1) as p1, \
                 tc.tile_pool(name="wp", bufs=3) as wp:
                XT = p1.tile([128, KC, T], F32R)
                COS = p1.tile([128, T], F32)
                SIN = p1.tile([128, T], F32)

                _engs = [nc.sync, nc.scalar, nc.gpsimd]

                # kc-major interleave: weight chunk then its x chunks, so
                # the kc-streaming Q projection consumes data on arrival
                wts = {}
                for wname in ("q", "k", "v"):
                    wts[wname] = wp.tile([128, KC, FPC], F32R, tag="w",
                                         name=f"wt_{wname}")
                for kc in range(KC):
                    nc.sync.dma_start(
                        out=wts["q"][:, kc, :],
                        in_=wq_d[128 * kc:128 * kc + 128, :])
                    nc.gpsimd.dma_start(
                        out=wts["k"][:, kc, :],
                        in_=wk_d[128 * kc:128 * kc + 128, :])
                    for tch in range(4):
                        eng = nc.sync if tch % 2 == 0 else nc.scalar
                        eng.dma_start(
                            out=XT[:, kc, 512 * tch:512 * tch + 512],
                            in_=xt_d[128 * kc:128 * kc + 128,
                                     512 * tch:512 * tch + 512])
                for kc in range(KC):
                    nc.gpsimd.dma_start(
                        out=wts["v"][:, kc, :],
                        in_=wv_d[128 * kc:128 * kc + 128, :])
                nc.scalar.dma_start(out=COS, in_=cos_d[:, :])
                nc.scalar.dma_start(out=SIN, in_=sin_d[:, :])

                def proj_qk(wt, dest, evict_eng, psqk, pfx):
                    # kc-streaming: 8 persistent accumulators (8 PSUM banks)
                    accs = [psqk.tile([128, 512], F32, tag=f"pq{i}",
                                      name=f"{pfx}acc{i}") for i in range(8)]
                    for kc in range(KC):
                        for dt_i in range(2):
                            for tch in range(4):
                                nc.tensor.matmul(
                                    accs[4 * dt_i + tch],
                                    wt[:, kc, 128 * dt_i:128 * dt_i + 128],
                                    XT[:, kc, 512 * tch:512 * tch + 512],
                                    start=(kc == 0), stop=(kc == KC - 1))
                    for dt_i in range(2):
                        for tch in range(4):
                            eng = (nc.vector.tensor_copy if tch % 2 == 0
                                   else nc.scalar.copy)
                            eng(
                                out=dest[dt_i][:, 512 * tch:512 * tch + 512],
                                in_=accs[4 * dt_i + tch])

                def rope(dest, pfx):
                    # in-place rope on the f32r projection output
                    for dt_i in range(2):
                        dst = dest[dt_i]
                        rot = p1.tile([128, T], F32R, tag=f"rot{dt_i}",
                                      name=f"{pfx}rot{dt_i}")
                        for g in range(2):
                            b0 = 64 * g
                            nc.gpsimd.dma_start(out=rot[b0:b0 + 32, :],
                                                in_=dst[b0 + 32:b0 + 64, :])
                            nc.gpsimd.dma_start(out=rot[b0 + 32:b0 + 64, :],
                                                in_=dst[b0:b0 + 32, :])
                        nc.gpsimd.tensor_mul(rot, rot, SIN)
                        nc.vector.tensor_mul(dst, dst, COS)
                        nc.vector.tensor_add(dst, dst, rot)

                with tc.tile_pool(name="psqk", bufs=1,
                                  space="PSUM") as psqk:
                    proj_qk(wts["q"], QT, nc.vector.tensor_copy, psqk, "q")
                    proj_qk(wts["k"], KT, nc.vector.tensor_copy, psqk, "k")
                    rope(QT, "q")
                    rope(KT, "k")

                # attention pools open early: S/exp for (h0,qh0) is
                # prefetched before the V projection to hide V evictions
                pss_cm = tc.tile_pool(name="pss", bufs=2, space="PSUM")
                pss = pss_cm.__enter__()

                def s_exp_piece(h, qh, kc):
                    qt, kt = QT[h // 2], KT[h // 2]
                    r0 = 64 * (h % 2)
                    q0, q1 = 1024 * qh, 1024 * qh + 1024
                    qs = max(q0, 128 * kc)
                    cols = q1 - qs
                    sp = pss.tile([128, 1024], F32, tag="sp")
                    off = 0
                    while off < cols:
                        # a matmul must not cross a 512-f32 PSUM bank edge
                        cw = min(512 - (off % 512), cols - off)
                        nc.tensor.matmul(
                            sp[:, off:off + cw],
                            kt[r0:r0 + 64, 128 * kc:128 * kc + 128],
                            qt[r0:r0 + 64, qs + off:qs + off + cw],
                            start=True, stop=True)
                        off += cw
                    pt = ptp.tile([128, 1024], F32R, tag="pt")
                    nc.scalar.activation(
                        out=pt[:, 0:cols], in_=sp[:, 0:cols],
                        func=EXP, scale=CON[:, 0:1])
                    if 128 * kc >= q0:  # diagonal block leads piece
                        nc.vector.tensor_mul(
                            pt[:, 0:128], pt[:, 0:128], MSK)
                    return pt, qs, cols

                def pv_piece(yaug, h, qh, kc, pt, qs, cols):
                    q0 = 1024 * qh
                    off = 0
                    while off < cols:
                        # PV chunks aligned to 512-windows so each window's
                        # PSUM accumulation group is clean
                        cw = min(512 - ((qs + off) % 512), cols - off)
                        w = (qs + off) // 512
                        nc.tensor.matmul(
                            yaug[:, qs - q0 + off:qs - q0 + off + cw],
                            VA[:, kc, 65 * h:65 * h + 65],
                            pt[:, off:off + cw],
                            start=(kc == 0), stop=(kc == 4 * w + 3))
                        off += cw

                pre_pts = [s_exp_piece(0, 0, kc) for kc in range(4)]

                # V projection -> VA [keys, 4*(64+ones)]
                wtv = wts["v"]
                with tc.tile_pool(name="psv", bufs=4, space="PSUM") as psv:
                    for t16 in range(16):
                        acc = psv.tile([128, FPC], F32, tag="pv")
                        for kc in range(KC):
                            nc.tensor.matmul(
                                acc,
                                XT[:, kc, 128 * t16:128 * t16 + 128],
                                wtv[:, kc, :],
                                start=(kc == 0), stop=(kc == KC - 1))
                        veng = (nc.vector.tensor_copy if t16 % 2 == 0
                                else nc.scalar.copy)
                        veng(
                            out=VA[:, t16, :].rearrange(
                                "p (h e) -> p h e", e=65)[:, :, 0:64],
                            in_=acc.rearrange("p (h e) -> p h e", e=64))

            # ------- Phase 2: attention, q-halved for tail overlap -------
            atp_cm = tc.tile_pool(name="atp", bufs=1)
            atp = atp_cm.__enter__()
            AT = [atp.tile([128, T], F32R, tag=f"at{i}", name=f"at{i}")
                  for i in range(2)]
            WOC = atp.tile([128, 2, D], F32R)
            for ft in range(2):
                nc.sync.dma_start(out=WOC[:, ft, :],
                                  in_=wo_d[128 * ft:128 * ft + 128, :])
            with tc.tile_pool(name="p2", bufs=2) as p2, \
                 tc.tile_pool(name="rb", bufs=2) as rbp, \
                 tc.tile_pool(name="psy", bufs=2, space="PSUM") as psy:
                for h in (0, 1, 3, 2):
                    for qh in range(2):  # q half: [1024*qh, 1024*qh+1024)
                        q0, q1 = 1024 * qh, 1024 * qh + 1024
                        yaug = psy.tile([65, 1024], F32, tag="yaug")
                        for kc in range(8 * (qh + 1)):
                            if h == 0 and qh == 0 and kc < 4:
                                pt, qs, cols = pre_pts[kc]
                            else:
                                pt, qs, cols = s_exp_piece(h, qh, kc)
                            pv_piece(yaug, h, qh, kc, pt, qs, cols)
                        rec = p2.tile([1, 1024], F32, tag="rec")
                        nc.vector.reciprocal(out=rec, in_=yaug[64:65, :])
                        wr_i = nc.sync.dma_start(out=rec_d[h, q0:q1],
                                                 in_=rec)
                        rb = rbp.tile([64, 1024], F32, tag="rb")
                        rsrc = rec_d[h, q0:q1]
                        rd_i = nc.sync.dma_start(
                            out=rb,
                            in_=bass.AP(tensor=rsrc.tensor,
                                        offset=rsrc.offset,
                                        ap=[[0, 64]] + list(rsrc.ap)))
                        # Tile does not track DRAM scratch RAW deps
                        add_dep_helper(rd_i.ins, wr_i.ins, sync=True,
                                       reason="recd bounce RAW")
                        if h % 2 == 0:
                            nc.vector.tensor_mul(
                                AT[h // 2][0:64, q0:q1], yaug[0:64, :], rb)
                        else:
                            stg = p2.tile([64, 1024], F32R, tag="stg")
                            nc.vector.tensor_mul(stg, yaug[0:64, :], rb)
                            nc.sync.dma_start(
                                out=AT[h // 2][64:128, q0:q1], in_=stg)

            # ---------------- Phase 3: output projection ----------------
            with tc.tile_pool(name="p3", bufs=3) as p3, \
                 tc.tile_pool(name="pso", bufs=2, space="PSUM") as pso:
                for t16 in range(16):
                    yo = pso.tile([128, D], F32, tag="yo")
                    for half in range(2):
                        for ft in range(2):
                            nc.tensor.matmul(
                                yo[:, 512 * half:512 * half + 512],
                                AT[ft][:, 128 * t16:128 * t16 + 128],
                                WOC[:, ft, 512 * half:512 * half + 512],
                                start=(ft == 0), stop=(ft == 1))
                    ot = p3.tile([128, D], F32, tag="ot")
                    nc.scalar.activation(out=ot, in_=yo, func=CPY,
                                         scale=CON[:, 1:2])
                    nc.sync.dma_start(
                        out=yp_d[128 * t16:128 * t16 + 128, :], in_=ot)
            atp_cm.__exit__(None, None, None)
            ptp_cm.__exit__(None, None, None)
            pss_cm.__exit__(None, None, None)

    _split_excess_waits(nc)
    return nc


_NC = None
_LAST_INMAPS = None


def _get_nc():
    global _NC
    if _NC is None:
        _NC = _build()
    return _NC


def _ternary_signs(w):
    """Mirror reference ternary_weight: returns (signs in {-1,0,1}, scale)."""
    try:
        import jax
        import jax.numpy as jnp
        cpu = jax.devices("cpu")[0]
        with jax.default_device(cpu):
            wj = jnp.asarray(np.asarray(w, dtype=np.float32))
            scale = jnp.mean(jnp.abs(wj))
            signs = jnp.round(jnp.clip(wj / (scale + 1e-8), -1.0, 1.0))
            return np.asarray(signs, dtype=np.float32), float(scale)
    except Exception:
        w = np.asarray(w, dtype=np.float32)
        scale = np.float32(np.mean(np.abs(w)))
        signs = np.round(np.clip(w / (scale + np.float32(1e-8)), -1.0, 1.0))
        return signs.astype(np.float32), float(scale)


def _round12(a):
    """Round fp32 to 12 mantissa bits (representable in f32r)."""
    u = np.ascontiguousarray(a, dtype=np.float32).view(np.uint32)
    r = (u + np.uint32(1 << 10)) & np.uint32(0xFFFFF800)
    return r.view(np.float32)


def _rope_tables():
    inv = (1.0 / (10000.0 ** (np.arange(0, HD, 2, dtype=np.float32) / HD))
           ).astype(np.float32)                      # [32]
    t = np.arange(T, dtype=np.float32)
    fr = np.outer(t, inv).astype(np.float32)         # [T, 32]
    cos1 = np.cos(fr).astype(np.float32)             # [T, 32]
    sin1 = np.sin(fr).astype(np.float32)
    # rows: d in 0..63 (freq d%32), tiled for 2 heads -> 128 rows
    cosd = np.concatenate([cos1, cos1], axis=1).T    # [64, T]
    sind = np.concatenate([sin1, sin1], axis=1).T    # [64, T]
    sgn = np.ones((HD, 1), dtype=np.float32)
    sgn[:HD // 2] = -1.0
    cos2 = np.tile(cosd, (2, 1)).astype(np.float32)          # [128, T]
    sins = np.tile(sind * sgn, (2, 1)).astype(np.float32)    # [128, T]
    return cos2, sins


def kernel(x, Wq, Wk, Wv, Wo, mask):
    global _LAST_INMAPS
    x = np.asarray(x, dtype=np.float32)
    mask = np.asarray(mask)
    assert np.array_equal(
        np.asarray(mask[0, 0], dtype=np.int32),
        np.tril(np.ones((T, T), dtype=np.int32))), "non-causal mask"

    qs, sq = _ternary_signs(Wq)
    ks, sk = _ternary_signs(Wk)
    vs, sv = _ternary_signs(Wv)
    os_, so = _ternary_signs(Wo)
    cos2, sins = _rope_tables()
    mvals = np.triu(np.ones((128, 128), dtype=np.float32))  # valid: i <= j
    consts = np.zeros((128, 2), dtype=np.float32)
    consts[:, 0] = np.float32(sq) * np.float32(sk) * np.float32(0.125)
    consts[:, 1] = np.float32(sv) * np.float32(so)

    in_maps = []
    for c in range(NCORES):
        b, g = c // 4, c % 4
        fsl = slice(FPC * g, FPC * g + FPC)
        in_maps.append({
            "xt": _round12(x[b].T),
            "wqt": np.ascontiguousarray(qs[fsl].T),
            "wkt": np.ascontiguousarray(ks[fsl].T),
            "wvt": np.ascontiguousarray(vs[fsl].T),
            "woc": np.ascontiguousarray(os_[:, fsl].T),
            "cos2": cos2,
            "sins": sins,
            "maskm": mvals,
            "consts": consts,
        })
    _LAST_INMAPS = in_maps

    res = run_bass_kernel_spmd(_get_nc(), in_maps,
                               core_ids=list(range(NCORES)))
    out = np.zeros((B, T, D), dtype=np.float32)
    for b in range(B):
        acc = np.zeros((T, D), dtype=np.float32)
        for g in range(4):
            acc += res.results[4 * b + g]["yp"]
        out[b] = acc
    return out


def bench(trace=True):
    """Re-run last inputs with NTFF tracing; returns BassKernelResults."""
    assert _LAST_INMAPS is not None, "call kernel() first"
    return run_bass_kernel_spmd(_get_nc(), _LAST_INMAPS,
                                core_ids=list(range(NCORES)), trace=trace)
